# revision 4
# baseline (speedup 1.0000x reference)
"""Trainium2 Bass kernel for ContrastHead (softnn contrastive KNN loss).

Data-parallel over points: 12500 points/core on 8 cores. The gather is the
bottleneck (Q7 SWDGE descriptor generation, ~8ns/desc), so the table packs
FOUR points per 1024B row (25000 rows -> int16-indexable in one window):
one dma_gather (3968 int16 idxs, elem 1024B) per 128-point tile instead of
the 4-window OR-merge scheme -> 4x fewer descriptors. The needed quarter is
selected on DVE with host-precomputed one-hot masks applied to per-quarter
dot products. Posmask and the point mask are label-only and computed on
host. Per tile: m4 = g4*f (128,31,4,64) f16, tree-add -> d4 (128,31,4);
d4 = s_j - 2*d4; one-hot select + s_i -> dist^2. Phase 2 as before on
(128, 3038). Host sums the 8x(128,2) outputs and divides.
"""

import numpy as np

import concourse.bacc as bacc
import concourse.bass as bass
import concourse.mybir as mybir
import concourse.tile as tile
from concourse import bass_utils

F16 = mybir.dt.float16
F32 = mybir.dt.float32
I16 = mybir.dt.int16

N = 100000
K = 31
C = 64
QROW = 128                  # f16 slots per point-quarter (256B)
ROW4 = 4 * QROW             # 512 f16 slots per packed table row (1024B)
TROWS = N // 4              # 25000 packed rows
SROW = 72                   # f16 slots per self row (144B)
NCORES = 8
PTS = N // NCORES           # 12500
TPC = (PTS + 127) // 128    # 98 tiles/core
NIDX = K * 128              # 3968 gather slots per tile
NIB16 = NIDX // 16          # 248
TEMP = 0.1
EPS = 1e-8

_CACHE = {}


def _build():
    nc = bacc.Bacc("TRN2", target_bir_lowering=False, debug=False)

    tabT = nc.dram_tensor("tab", (TROWS, ROW4), F16, kind="ExternalInput")
    selfT = nc.dram_tensor("selftab", (128, TPC, SROW), F16, kind="ExternalInput")
    idxT = nc.dram_tensor("nidx16", (TPC, 128, NIB16), I16, kind="ExternalInput")
    ohT = nc.dram_tensor("oh", (128, TPC, K, 4), F32, kind="ExternalInput")
    pmT = nc.dram_tensor("pm", (128, TPC, K), F32, kind="ExternalInput")
    maT = nc.dram_tensor("ma", (128, TPC), F32, kind="ExternalInput")
    outT = nc.dram_tensor("out", (128, 2), F32, kind="ExternalOutput")

    with tile.TileContext(nc) as tc:
        with (
            tc.tile_pool(name="res", bufs=1) as res,
            tc.tile_pool(name="gpool", bufs=2) as gpool,
            tc.tile_pool(name="ipool", bufs=4) as ipool,
            tc.tile_pool(name="mpool", bufs=2) as mpool,
            tc.tile_pool(name="dpool", bufs=2) as dpool,
            tc.tile_pool(name="p2", bufs=1) as p2,
        ):
            selfsb = res.tile([128, TPC, SROW], F16)
            nc.sync.dma_start(out=selfsb[:], in_=selfT.ap())
            ohsb = res.tile([128, TPC, K, 4], F32)
            nc.sync.dma_start(out=ohsb[:], in_=ohT.ap())
            pmsb = res.tile([128, TPC, K], F32)
            nc.sync.dma_start(out=pmsb[:], in_=pmT.ap())
            masb = res.tile([128, TPC], F32)
            nc.sync.dma_start(out=masb[:], in_=maT.ap())

            dist2 = res.tile([128, TPC, K], F32)

            selff32 = selfsb[:].bitcast(F32)        # (128, TPC, 36)

            for t in range(TPC):
                iv = ipool.tile([128, NIB16], I16, tag="idx")
                nc.sync.dma_start(out=iv[:], in_=idxT.ap()[t])
                g = gpool.tile([128, K, ROW4], F16, tag="g")
                nc.gpsimd.dma_gather(
                    out_ap=g[:],
                    in_ap=tabT.ap(),
                    idxs_ap=iv[:],
                    num_idxs=NIDX,
                    num_idxs_reg=NIDX,
                    elem_size=ROW4,
                    single_packet=False,
                )
                g4 = g[:].rearrange("p j (q c) -> p j q c", q=4)   # (128,K,4,QROW)
                m4 = mpool.tile([128, K, 4, C], F16, tag="m")
                fb = (
                    selfsb[:, t, 0:C]
                    .unsqueeze(1)
                    .unsqueeze(2)
                    .broadcast_to([128, K, 4, C])
                )
                nc.vector.tensor_tensor(
                    out=m4[:], in0=g4[:, :, :, 0:C], in1=fb,
                    op=mybir.AluOpType.mult,
                )
                nc.vector.tensor_add(
                    out=m4[:, :, :, 0:32], in0=m4[:, :, :, 0:32],
                    in1=m4[:, :, :, 32:64],
                )
                nc.vector.tensor_add(
                    out=m4[:, :, :, 0:16], in0=m4[:, :, :, 0:16],
                    in1=m4[:, :, :, 16:32],
                )
                nc.vector.tensor_add(
                    out=m4[:, :, :, 0:8], in0=m4[:, :, :, 0:8],
                    in1=m4[:, :, :, 8:16],
                )
                d4 = dpool.tile([128, K, 4], F32, tag="d")
                nc.vector.reduce_sum(
                    out=d4[:], in_=m4[:, :, :, 0:8], axis=mybir.AxisListType.X
                )
                # d4 = -2*dot_q + s_jq  (s at f32 slot 32 of each quarter)
                gf32 = g[:].bitcast(F32).rearrange(
                    "p j (q c) -> p j q c", q=4
                )                                    # (128,K,4,64) f32
                s4 = gf32[:, :, :, 32]               # (128,K,4)
                nc.vector.scalar_tensor_tensor(
                    out=d4[:], in0=d4[:], scalar=-2.0, in1=s4,
                    op0=mybir.AluOpType.mult, op1=mybir.AluOpType.add,
                )
                # one-hot select the real quarter
                nc.vector.tensor_tensor(
                    out=d4[:], in0=d4[:], in1=ohsb[:, t],
                    op=mybir.AluOpType.mult,
                )
                nc.vector.tensor_add(
                    out=d4[:, :, 0:2], in0=d4[:, :, 0:2], in1=d4[:, :, 2:4]
                )
                nc.vector.tensor_add(
                    out=d4[:, :, 0], in0=d4[:, :, 0], in1=d4[:, :, 1]
                )
                # dist2 = sel + s_i
                si = selff32[:, t, 32].unsqueeze(1).broadcast_to([128, K])
                nc.vector.tensor_add(
                    out=dist2[:, t, :], in0=d4[:, :, 0], in1=si
                )

            # ---- phase 2 ----
            nc.scalar.sqrt(out=dist2[:], in_=dist2[:])
            mind = p2.tile([128, TPC], F32)
            nc.vector.tensor_reduce(
                out=mind[:], in_=dist2[:], axis=mybir.AxisListType.X,
                op=mybir.AluOpType.min,
            )
            mbc = mind[:].unsqueeze(2).broadcast_to([128, TPC, K])
            nc.vector.tensor_tensor(
                out=dist2[:], in0=dist2[:], in1=mbc, op=mybir.AluOpType.subtract
            )
            nc.scalar.activation(
                out=dist2[:], in_=dist2[:],
                func=mybir.ActivationFunctionType.Exp, scale=-1.0 / TEMP,
            )
            negs = p2.tile([128, TPC], F32)
            nc.vector.reduce_sum(out=negs[:], in_=dist2[:], axis=mybir.AxisListType.X)
            nc.vector.tensor_tensor(
                out=dist2[:], in0=dist2[:], in1=pmsb[:], op=mybir.AluOpType.mult
            )
            poss = p2.tile([128, TPC], F32)
            nc.vector.reduce_sum(out=poss[:], in_=dist2[:], axis=mybir.AxisListType.X)
            rn = p2.tile([128, TPC], F32)
            nc.vector.reciprocal(out=rn[:], in_=negs[:])
            ratio = p2.tile([128, TPC], F32)
            nc.vector.tensor_tensor(
                out=ratio[:], in0=poss[:], in1=rn[:], op=mybir.AluOpType.mult
            )
            eps_t = p2.tile([128, 1], F32)
            nc.vector.memset(eps_t[:], EPS)
            lg = p2.tile([128, TPC], F32)
            nc.scalar.activation(
                out=lg[:], in_=ratio[:],
                func=mybir.ActivationFunctionType.Ln, bias=eps_t[:],
            )
            nc.vector.tensor_tensor(
                out=lg[:], in0=lg[:], in1=masb[:], op=mybir.AluOpType.mult
            )
            outsb = p2.tile([128, 2], F32)
            nc.vector.reduce_sum(out=outsb[:, 0:1], in_=lg[:], axis=mybir.AxisListType.X)
            nc.vector.reduce_sum(out=outsb[:, 1:2], in_=masb[:], axis=mybir.AxisListType.X)
            nc.sync.dma_start(out=outT.ap(), in_=outsb[:])

    nc.compile()
    return nc


def _get_nc():
    if "nc" not in _CACHE:
        _CACHE["nc"] = _build()
    return _CACHE["nc"]


def _pack_table(features: np.ndarray) -> tuple[np.ndarray, np.ndarray]:
    """packed (TROWS, 512) f16 table + per-point f32 norms."""
    f16 = features.astype(np.float16)
    s = np.sum(features.astype(np.float64) ** 2, axis=1).astype(np.float32)
    q = np.zeros((N, QROW), dtype=np.float16)
    q[:, 0:C] = f16
    q[:, 64:66] = s[:, None].view(np.float16)
    tab = q.reshape(TROWS, ROW4)
    return tab, s


def _core_inputs(tab, s, features, labels, neighbor_idx, lo, hi):
    npts = hi - lo
    pad = TPC * 128

    selfrows = np.zeros((pad, SROW), dtype=np.float16)
    selfrows[:npts, 0:C] = features[lo:hi].astype(np.float16)
    selfrows[:npts, 64:66] = s[lo:hi, None].view(np.float16)

    nidx_c = np.zeros((pad, K), dtype=np.int64)
    nidx_c[:npts] = neighbor_idx[lo:hi]
    pm_c = np.zeros((pad, K), dtype=np.float32)
    pm_c[:npts] = (labels[lo:hi, None] == labels[neighbor_idx[lo:hi]]).astype(
        np.float32
    )
    cnt = pm_c.sum(axis=1)
    ma_c = ((cnt > 0) & (cnt < K)).astype(np.float32)
    ma_c[npts:] = 0.0

    # one-hot quarter masks
    qsel = (nidx_c & 3).astype(np.int64)                       # (pad, K)
    oh_c = (qsel[:, :, None] == np.arange(4)[None, None, :]).astype(np.float32)

    # gather indices: packed row = nidx >> 2, flat slot order j*128 + p
    idx4 = (nidx_c >> 2).astype(np.int16)                      # (pad, K)
    nidx_pm = idx4.reshape(TPC, 128, K).transpose(1, 0, 2)     # (128, TPC, K)
    flat = nidx_pm.transpose(1, 2, 0).reshape(TPC, NIDX)       # (TPC, 3968)
    wrapped = flat.reshape(TPC, NIB16, 16).transpose(0, 2, 1)  # (TPC, 16, NIB16)
    nidx16 = np.ascontiguousarray(np.tile(wrapped, (1, 8, 1)))

    def pmaj(x):
        return np.ascontiguousarray(
            x.reshape(TPC, 128, *x.shape[1:]).transpose(
                (1, 0) + tuple(range(2, x.ndim + 1))
            )
        )

    return {
        "tab": tab,
        "selftab": pmaj(selfrows),
        "nidx16": nidx16,
        "oh": pmaj(oh_c),
        "pm": pmaj(pm_c),
        "ma": pmaj(ma_c),
    }


def run(features, labels, neighbor_idx, trace=False):
    nc = _get_nc()
    tab, s = _pack_table(features)
    in_maps = [
        _core_inputs(tab, s, features, labels, neighbor_idx, c * PTS, (c + 1) * PTS)
        for c in range(NCORES)
    ]
    res = bass_utils.run_bass_kernel_spmd(
        nc, in_maps, core_ids=list(range(NCORES)), trace=trace
    )
    tot = 0.0
    ccnt = 0.0
    for o in res.results:
        tot += float(o["out"][:, 0].astype(np.float64).sum())
        ccnt += float(o["out"][:, 1].astype(np.float64).sum())
    loss = np.float32(-tot / max(ccnt, 1.0))
    return loss, res


def kernel(features, labels, neighbor_idx):
    loss, _ = run(features, labels, neighbor_idx, trace=False)
    return loss


# revision 5
# speedup vs baseline: 1.2555x; 1.2555x over previous
"""Trainium2 Bass kernel for ContrastHead (softnn contrastive KNN loss).

Data-parallel over points: 12500 points/core on 8 cores. The gather is the
bottleneck (Q7 SWDGE descriptor generation, ~8ns/desc), so the table packs
FOUR points per 1024B row (25000 rows -> int16-indexable in one window):
one dma_gather (3968 int16 idxs, elem 1024B) per 128-point tile instead of
the 4-window OR-merge scheme -> 4x fewer descriptors. The needed quarter is
selected on DVE with host-precomputed one-hot masks applied to per-quarter
dot products. Posmask and the point mask are label-only and computed on
host. Per tile: m4 = g4*f (128,31,4,64) f16, tree-add -> d4 (128,31,4);
d4 = s_j - 2*d4; one-hot select + s_i -> dist^2. Phase 2 as before on
(128, 3038). Host sums the 8x(128,2) outputs and divides.
"""

import numpy as np

import concourse.bacc as bacc
import concourse.bass as bass
import concourse.mybir as mybir
import concourse.tile as tile
from concourse import bass_utils

F16 = mybir.dt.float16
F32 = mybir.dt.float32
I16 = mybir.dt.int16

N = 100000
K = 31
C = 64
QROW = 128                  # f16 slots per point-quarter (256B)
ROW4 = 4 * QROW             # 512 f16 slots per packed table row (1024B)
TROWS = N // 4              # 25000 packed rows
SROW = 72                   # f16 slots per self row (144B)
NCORES = 8
PTS = N // NCORES           # 12500
TPC = (PTS + 127) // 128    # 98 tiles/core
NIDX = K * 128              # 3968 gather slots per tile
NIB16 = NIDX // 16          # 248
TEMP = 0.1
EPS = 1e-8

_CACHE = {}


def _build():
    nc = bacc.Bacc(
        "TRN2", target_bir_lowering=False, debug=False, num_swdge_queues=4
    )

    tabT = nc.dram_tensor("tab", (TROWS, ROW4), F16, kind="ExternalInput")
    selfT = nc.dram_tensor("selftab", (128, TPC, SROW), F16, kind="ExternalInput")
    idxT = nc.dram_tensor("nidx16", (TPC, 128, NIB16), I16, kind="ExternalInput")
    ohT = nc.dram_tensor("oh", (128, TPC, K, 4), F32, kind="ExternalInput")
    pmT = nc.dram_tensor("pm", (128, TPC, K), F32, kind="ExternalInput")
    maT = nc.dram_tensor("ma", (128, TPC), F32, kind="ExternalInput")
    outT = nc.dram_tensor("out", (128, 2), F32, kind="ExternalOutput")

    with tile.TileContext(nc) as tc:
        with (
            tc.tile_pool(name="res", bufs=1) as res,
            tc.tile_pool(name="gpool", bufs=2) as gpool,
            tc.tile_pool(name="ipool", bufs=4) as ipool,
            tc.tile_pool(name="mpool", bufs=2) as mpool,
            tc.tile_pool(name="dpool", bufs=2) as dpool,
            tc.tile_pool(name="p2", bufs=1) as p2,
        ):
            # big resident loads go on the scalar HWDGE queue so the first
            # per-tile idx DMAs (sync queue) aren't stuck behind them
            selfsb = res.tile([128, TPC, SROW], F16)
            nc.scalar.dma_start(out=selfsb[:], in_=selfT.ap())
            ohsb = res.tile([128, TPC, K, 4], F32)
            nc.scalar.dma_start(out=ohsb[:], in_=ohT.ap())
            pmsb = res.tile([128, TPC, K], F32)
            nc.scalar.dma_start(out=pmsb[:], in_=pmT.ap())
            masb = res.tile([128, TPC], F32)
            nc.scalar.dma_start(out=masb[:], in_=maT.ap())

            dist2 = res.tile([128, TPC, K], F32)

            selff32 = selfsb[:].bitcast(F32)        # (128, TPC, 36)

            def phase2_chunk(c0, c1, mind, negs, poss):
                d2c = dist2[:, c0:c1]
                nc.scalar.sqrt(out=d2c, in_=d2c)
                nc.vector.tensor_reduce(
                    out=mind[:, c0:c1], in_=d2c, axis=mybir.AxisListType.X,
                    op=mybir.AluOpType.min,
                )
                mbc = (
                    mind[:, c0:c1].unsqueeze(2).broadcast_to([128, c1 - c0, K])
                )
                nc.vector.tensor_tensor(
                    out=d2c, in0=d2c, in1=mbc, op=mybir.AluOpType.subtract
                )
                nc.scalar.activation(
                    out=d2c, in_=d2c,
                    func=mybir.ActivationFunctionType.Exp, scale=-1.0 / TEMP,
                )
                nc.vector.reduce_sum(
                    out=negs[:, c0:c1], in_=d2c, axis=mybir.AxisListType.X
                )
                nc.vector.tensor_tensor(
                    out=d2c, in0=d2c, in1=pmsb[:, c0:c1],
                    op=mybir.AluOpType.mult,
                )
                nc.vector.reduce_sum(
                    out=poss[:, c0:c1], in_=d2c, axis=mybir.AxisListType.X
                )

            mind = p2.tile([128, TPC], F32)
            negs = p2.tile([128, TPC], F32)
            poss = p2.tile([128, TPC], F32)
            chunks = [0, 60, 90, TPC]

            for t in range(TPC):
                iv = ipool.tile([128, NIB16], I16, tag="idx")
                nc.sync.dma_start(out=iv[:], in_=idxT.ap()[t])
                g = gpool.tile([128, K, ROW4], F16, tag="g")
                nc.gpsimd.dma_gather(
                    out_ap=g[:],
                    in_ap=tabT.ap(),
                    idxs_ap=iv[:],
                    num_idxs=NIDX,
                    num_idxs_reg=NIDX,
                    elem_size=ROW4,
                    single_packet=False,
                    queue_num=t % 4,
                )
                g4 = g[:].rearrange("p j (q c) -> p j q c", q=4)   # (128,K,4,QROW)
                m4 = mpool.tile([128, K, 4, C], F16, tag="m")
                fb = (
                    selfsb[:, t, 0:C]
                    .unsqueeze(1)
                    .unsqueeze(2)
                    .broadcast_to([128, K, 4, C])
                )
                nc.vector.tensor_tensor(
                    out=m4[:], in0=g4[:, :, :, 0:C], in1=fb,
                    op=mybir.AluOpType.mult,
                )
                d4 = dpool.tile([128, K, 4], F32, tag="d")
                nc.vector.reduce_sum(
                    out=d4[:], in_=m4[:], axis=mybir.AxisListType.X
                )
                # d4 = -2*dot_q + s_jq  (s at f32 slot 32 of each quarter)
                gf32 = g[:].bitcast(F32).rearrange(
                    "p j (q c) -> p j q c", q=4
                )                                    # (128,K,4,64) f32
                s4 = gf32[:, :, :, 32]               # (128,K,4)
                nc.vector.scalar_tensor_tensor(
                    out=d4[:], in0=d4[:], scalar=-2.0, in1=s4,
                    op0=mybir.AluOpType.mult, op1=mybir.AluOpType.add,
                )
                # one-hot select the real quarter
                nc.vector.tensor_tensor(
                    out=d4[:], in0=d4[:], in1=ohsb[:, t],
                    op=mybir.AluOpType.mult,
                )
                nc.vector.tensor_add(
                    out=d4[:, :, 0:2], in0=d4[:, :, 0:2], in1=d4[:, :, 2:4]
                )
                nc.vector.tensor_add(
                    out=d4[:, :, 0], in0=d4[:, :, 0], in1=d4[:, :, 1]
                )
                # dist2 = sel + s_i
                si = selff32[:, t, 32].unsqueeze(1).broadcast_to([128, K])
                nc.vector.tensor_add(
                    out=dist2[:, t, :], in0=d4[:, :, 0], in1=si
                )
                # interleave finished phase-2 chunks under the later gathers
                for ci in range(len(chunks) - 2):
                    if t == chunks[ci + 1] - 1 + 2:  # 2 tiles of slack
                        phase2_chunk(chunks[ci], chunks[ci + 1], mind, negs, poss)

            # ---- phase 2 (final chunk + epilogue) ----
            phase2_chunk(chunks[-2], chunks[-1], mind, negs, poss)
            rn = p2.tile([128, TPC], F32)
            nc.vector.reciprocal(out=rn[:], in_=negs[:])
            ratio = p2.tile([128, TPC], F32)
            nc.vector.tensor_tensor(
                out=ratio[:], in0=poss[:], in1=rn[:], op=mybir.AluOpType.mult
            )
            eps_t = p2.tile([128, 1], F32)
            nc.vector.memset(eps_t[:], EPS)
            lg = p2.tile([128, TPC], F32)
            nc.scalar.activation(
                out=lg[:], in_=ratio[:],
                func=mybir.ActivationFunctionType.Ln, bias=eps_t[:],
            )
            nc.vector.tensor_tensor(
                out=lg[:], in0=lg[:], in1=masb[:], op=mybir.AluOpType.mult
            )
            outsb = p2.tile([128, 2], F32)
            nc.vector.reduce_sum(out=outsb[:, 0:1], in_=lg[:], axis=mybir.AxisListType.X)
            nc.vector.reduce_sum(out=outsb[:, 1:2], in_=masb[:], axis=mybir.AxisListType.X)
            nc.sync.dma_start(out=outT.ap(), in_=outsb[:])

    nc.compile()
    return nc


def _get_nc():
    if "nc" not in _CACHE:
        _CACHE["nc"] = _build()
    return _CACHE["nc"]


def _pack_table(features: np.ndarray) -> tuple[np.ndarray, np.ndarray]:
    """packed (TROWS, 512) f16 table + per-point f32 norms."""
    f16 = features.astype(np.float16)
    s = np.sum(features.astype(np.float64) ** 2, axis=1).astype(np.float32)
    q = np.zeros((N, QROW), dtype=np.float16)
    q[:, 0:C] = f16
    q[:, 64:66] = s[:, None].view(np.float16)
    tab = q.reshape(TROWS, ROW4)
    return tab, s


def _core_inputs(tab, s, features, labels, neighbor_idx, lo, hi):
    npts = hi - lo
    pad = TPC * 128

    selfrows = np.zeros((pad, SROW), dtype=np.float16)
    selfrows[:npts, 0:C] = features[lo:hi].astype(np.float16)
    selfrows[:npts, 64:66] = s[lo:hi, None].view(np.float16)

    nidx_c = np.zeros((pad, K), dtype=np.int64)
    nidx_c[:npts] = neighbor_idx[lo:hi]
    pm_c = np.zeros((pad, K), dtype=np.float32)
    pm_c[:npts] = (labels[lo:hi, None] == labels[neighbor_idx[lo:hi]]).astype(
        np.float32
    )
    cnt = pm_c.sum(axis=1)
    ma_c = ((cnt > 0) & (cnt < K)).astype(np.float32)
    ma_c[npts:] = 0.0

    # one-hot quarter masks
    qsel = (nidx_c & 3).astype(np.int64)                       # (pad, K)
    oh_c = (qsel[:, :, None] == np.arange(4)[None, None, :]).astype(np.float32)

    # gather indices: packed row = nidx >> 2, flat slot order j*128 + p
    idx4 = (nidx_c >> 2).astype(np.int16)                      # (pad, K)
    nidx_pm = idx4.reshape(TPC, 128, K).transpose(1, 0, 2)     # (128, TPC, K)
    flat = nidx_pm.transpose(1, 2, 0).reshape(TPC, NIDX)       # (TPC, 3968)
    wrapped = flat.reshape(TPC, NIB16, 16).transpose(0, 2, 1)  # (TPC, 16, NIB16)
    nidx16 = np.ascontiguousarray(np.tile(wrapped, (1, 8, 1)))

    def pmaj(x):
        return np.ascontiguousarray(
            x.reshape(TPC, 128, *x.shape[1:]).transpose(
                (1, 0) + tuple(range(2, x.ndim + 1))
            )
        )

    return {
        "tab": tab,
        "selftab": pmaj(selfrows),
        "nidx16": nidx16,
        "oh": pmaj(oh_c),
        "pm": pmaj(pm_c),
        "ma": pmaj(ma_c),
    }


def run(features, labels, neighbor_idx, trace=False):
    nc = _get_nc()
    tab, s = _pack_table(features)
    in_maps = [
        _core_inputs(tab, s, features, labels, neighbor_idx, c * PTS, (c + 1) * PTS)
        for c in range(NCORES)
    ]
    res = bass_utils.run_bass_kernel_spmd(
        nc, in_maps, core_ids=list(range(NCORES)), trace=trace
    )
    tot = 0.0
    ccnt = 0.0
    for o in res.results:
        tot += float(o["out"][:, 0].astype(np.float64).sum())
        ccnt += float(o["out"][:, 1].astype(np.float64).sum())
    loss = np.float32(-tot / max(ccnt, 1.0))
    return loss, res


def kernel(features, labels, neighbor_idx):
    loss, _ = run(features, labels, neighbor_idx, trace=False)
    return loss


# revision 6
# speedup vs baseline: 1.9478x; 1.5515x over previous
"""Trainium2 Bass kernel for ContrastHead (softnn contrastive KNN loss).

Data-parallel over points: 12500 points/core on 8 cores. The gather is the
bottleneck (Q7 SWDGE descriptor generation + SDMA drain), so the table packs
FOUR points' features per 512B row (25000 rows -> int16-indexable in one
window): one dma_gather (3968 int16 idxs, elem 512B) per 128-point tile,
round-robined over 4 SWDGE queues. dist^2 is computed directly as
sum((f_i - g)^2) per quarter (no norms needed); the real quarter is selected
with host-precomputed one-hot masks. Posmask and the point mask are
label-only and computed on host. Phase 2 runs in chunks interleaved under
the later gathers. Host sums the 8x(128,2) outputs and divides.
"""

import numpy as np

import concourse.bacc as bacc
import concourse.bass as bass
import concourse.mybir as mybir
import concourse.tile as tile
from concourse import bass_utils

F16 = mybir.dt.float16
F32 = mybir.dt.float32
I16 = mybir.dt.int16

N = 100000
K = 31
C = 64
ROW4 = 4 * C                # 256 f16 slots per packed table row (512B)
TROWS = N // 4              # 25000 packed rows
NCORES = 8
PTS = N // NCORES           # 12500
TPC = (PTS + 127) // 128    # 98 tiles/core
NIDX = K * 128              # 3968 gather slots per tile
NIB16 = NIDX // 16          # 248
TEMP = 0.1
EPS = 1e-8

_CACHE = {}


def _build():
    nc = bacc.Bacc(
        "TRN2", target_bir_lowering=False, debug=False, num_swdge_queues=4
    )

    tabT = nc.dram_tensor("tab", (TROWS, ROW4), F16, kind="ExternalInput")
    selfT = nc.dram_tensor("selftab", (128, TPC, C), F16, kind="ExternalInput")
    idxT = nc.dram_tensor("nidx16", (TPC, 128, NIB16), I16, kind="ExternalInput")
    ohT = nc.dram_tensor("oh", (128, TPC, K, 4), F32, kind="ExternalInput")
    pmT = nc.dram_tensor("pm", (128, TPC, K), F32, kind="ExternalInput")
    maT = nc.dram_tensor("ma", (128, TPC), F32, kind="ExternalInput")
    outT = nc.dram_tensor("out", (128, 2), F32, kind="ExternalOutput")

    with tile.TileContext(nc) as tc:
        with (
            tc.tile_pool(name="res", bufs=1) as res,
            tc.tile_pool(name="gpool", bufs=3) as gpool,
            tc.tile_pool(name="ipool", bufs=4) as ipool,
            tc.tile_pool(name="mpool", bufs=2) as mpool,
            tc.tile_pool(name="dpool", bufs=2) as dpool,
            tc.tile_pool(name="p2", bufs=1) as p2,
        ):
            # big resident loads go on the scalar HWDGE queue so the first
            # per-tile idx DMAs (sync queue) aren't stuck behind them
            selfsb = res.tile([128, TPC, C], F16)
            nc.scalar.dma_start(out=selfsb[:], in_=selfT.ap())
            ohsb = res.tile([128, TPC, K, 4], F32)
            nc.scalar.dma_start(out=ohsb[:], in_=ohT.ap())
            pmsb = res.tile([128, TPC, K], F32)
            nc.scalar.dma_start(out=pmsb[:], in_=pmT.ap())
            masb = res.tile([128, TPC], F32)
            nc.scalar.dma_start(out=masb[:], in_=maT.ap())

            dist2 = res.tile([128, TPC, K], F32)

            def phase2_chunk(c0, c1, mind, negs, poss):
                d2c = dist2[:, c0:c1]
                nc.scalar.sqrt(out=d2c, in_=d2c)
                nc.vector.tensor_reduce(
                    out=mind[:, c0:c1], in_=d2c, axis=mybir.AxisListType.X,
                    op=mybir.AluOpType.min,
                )
                mbc = (
                    mind[:, c0:c1].unsqueeze(2).broadcast_to([128, c1 - c0, K])
                )
                nc.vector.tensor_tensor(
                    out=d2c, in0=d2c, in1=mbc, op=mybir.AluOpType.subtract
                )
                nc.scalar.activation(
                    out=d2c, in_=d2c,
                    func=mybir.ActivationFunctionType.Exp, scale=-1.0 / TEMP,
                )
                nc.vector.reduce_sum(
                    out=negs[:, c0:c1], in_=d2c, axis=mybir.AxisListType.X
                )
                nc.vector.tensor_tensor(
                    out=d2c, in0=d2c, in1=pmsb[:, c0:c1],
                    op=mybir.AluOpType.mult,
                )
                nc.vector.reduce_sum(
                    out=poss[:, c0:c1], in_=d2c, axis=mybir.AxisListType.X
                )

            mind = p2.tile([128, TPC], F32)
            negs = p2.tile([128, TPC], F32)
            poss = p2.tile([128, TPC], F32)
            chunks = [0, 50, 80, 96, TPC]

            for t in range(TPC):
                iv = ipool.tile([128, NIB16], I16, tag="idx")
                nc.sync.dma_start(out=iv[:], in_=idxT.ap()[t])
                g = gpool.tile([128, K, ROW4], F16, tag="g")
                nc.gpsimd.dma_gather(
                    out_ap=g[:],
                    in_ap=tabT.ap(),
                    idxs_ap=iv[:],
                    num_idxs=NIDX,
                    num_idxs_reg=NIDX,
                    elem_size=ROW4,
                    single_packet=False,
                    queue_num=t % 4,
                )
                g4 = g[:].rearrange("p j (q c) -> p j q c", q=4)   # (128,K,4,C)
                m4 = mpool.tile([128, K, 4, C], F16, tag="m")
                fb = (
                    selfsb[:, t, :]
                    .unsqueeze(1)
                    .unsqueeze(2)
                    .broadcast_to([128, K, 4, C])
                )
                nc.vector.tensor_tensor(
                    out=m4[:], in0=g4, in1=fb, op=mybir.AluOpType.subtract
                )
                nc.vector.tensor_tensor(
                    out=m4[:], in0=m4[:], in1=m4[:], op=mybir.AluOpType.mult
                )
                d4 = dpool.tile([128, K, 4], F32, tag="d")
                nc.vector.reduce_sum(
                    out=d4[:], in_=m4[:], axis=mybir.AxisListType.X
                )
                # one-hot select the real quarter -> dist^2
                nc.vector.tensor_tensor(
                    out=d4[:], in0=d4[:], in1=ohsb[:, t],
                    op=mybir.AluOpType.mult,
                )
                nc.vector.tensor_add(
                    out=d4[:, :, 0:2], in0=d4[:, :, 0:2], in1=d4[:, :, 2:4]
                )
                nc.vector.tensor_add(
                    out=dist2[:, t, :], in0=d4[:, :, 0], in1=d4[:, :, 1]
                )
                # interleave finished phase-2 chunks under the later gathers
                for ci in range(len(chunks) - 2):
                    if t == chunks[ci + 1] + 1:
                        phase2_chunk(chunks[ci], chunks[ci + 1], mind, negs, poss)

            # ---- phase 2 (final chunk + epilogue) ----
            phase2_chunk(chunks[-2], chunks[-1], mind, negs, poss)
            rn = p2.tile([128, TPC], F32)
            nc.vector.reciprocal(out=rn[:], in_=negs[:])
            ratio = p2.tile([128, TPC], F32)
            nc.vector.tensor_tensor(
                out=ratio[:], in0=poss[:], in1=rn[:], op=mybir.AluOpType.mult
            )
            eps_t = p2.tile([128, 1], F32)
            nc.vector.memset(eps_t[:], EPS)
            lg = p2.tile([128, TPC], F32)
            nc.scalar.activation(
                out=lg[:], in_=ratio[:],
                func=mybir.ActivationFunctionType.Ln, bias=eps_t[:],
            )
            nc.vector.tensor_tensor(
                out=lg[:], in0=lg[:], in1=masb[:], op=mybir.AluOpType.mult
            )
            outsb = p2.tile([128, 2], F32)
            nc.vector.reduce_sum(out=outsb[:, 0:1], in_=lg[:], axis=mybir.AxisListType.X)
            nc.vector.reduce_sum(out=outsb[:, 1:2], in_=masb[:], axis=mybir.AxisListType.X)
            nc.sync.dma_start(out=outT.ap(), in_=outsb[:])

    nc.compile()
    return nc


def _get_nc():
    if "nc" not in _CACHE:
        _CACHE["nc"] = _build()
    return _CACHE["nc"]


def _core_inputs(tab, features, labels, neighbor_idx, lo, hi):
    npts = hi - lo
    pad = TPC * 128

    selfrows = np.zeros((pad, C), dtype=np.float16)
    selfrows[:npts] = features[lo:hi].astype(np.float16)

    nidx_c = np.zeros((pad, K), dtype=np.int64)
    nidx_c[:npts] = neighbor_idx[lo:hi]
    pm_c = np.zeros((pad, K), dtype=np.float32)
    pm_c[:npts] = (labels[lo:hi, None] == labels[neighbor_idx[lo:hi]]).astype(
        np.float32
    )
    cnt = pm_c.sum(axis=1)
    ma_c = ((cnt > 0) & (cnt < K)).astype(np.float32)
    ma_c[npts:] = 0.0

    qsel = (nidx_c & 3).astype(np.int64)                       # (pad, K)
    oh_c = (qsel[:, :, None] == np.arange(4)[None, None, :]).astype(np.float32)

    idx4 = (nidx_c >> 2).astype(np.int16)                      # (pad, K)
    nidx_pm = idx4.reshape(TPC, 128, K).transpose(1, 0, 2)     # (128, TPC, K)
    flat = nidx_pm.transpose(1, 2, 0).reshape(TPC, NIDX)       # (TPC, 3968)
    wrapped = flat.reshape(TPC, NIB16, 16).transpose(0, 2, 1)  # (TPC, 16, NIB16)
    nidx16 = np.ascontiguousarray(np.tile(wrapped, (1, 8, 1)))

    def pmaj(x):
        return np.ascontiguousarray(
            x.reshape(TPC, 128, *x.shape[1:]).transpose(
                (1, 0) + tuple(range(2, x.ndim + 1))
            )
        )

    return {
        "tab": tab,
        "selftab": pmaj(selfrows),
        "nidx16": nidx16,
        "oh": pmaj(oh_c),
        "pm": pmaj(pm_c),
        "ma": pmaj(ma_c),
    }


def run(features, labels, neighbor_idx, trace=False):
    nc = _get_nc()
    tab = features.astype(np.float16).reshape(TROWS, ROW4)
    in_maps = [
        _core_inputs(tab, features, labels, neighbor_idx, c * PTS, (c + 1) * PTS)
        for c in range(NCORES)
    ]
    res = bass_utils.run_bass_kernel_spmd(
        nc, in_maps, core_ids=list(range(NCORES)), trace=trace
    )
    tot = 0.0
    ccnt = 0.0
    for o in res.results:
        tot += float(o["out"][:, 0].astype(np.float64).sum())
        ccnt += float(o["out"][:, 1].astype(np.float64).sum())
    loss = np.float32(-tot / max(ccnt, 1.0))
    return loss, res


def kernel(features, labels, neighbor_idx):
    loss, _ = run(features, labels, neighbor_idx, trace=False)
    return loss
